# revision 1
# baseline (speedup 1.0000x reference)
"""Trainium2 Bass kernel for nn_ModalityMoERouter (expert-choice MoE routing).

Contract: kernel(**inputs) takes the FULL inputs from reference.setup_inputs()
and returns (dispatch, combine), each (16, 8192, 16) float32.

Sharding: data-parallel over batch B=16 across 8 NeuronCores (2 batches/core);
gate weights and expert centers replicated. The global mean(dists) scalar is
computed with one AllReduce (overlapped with the gate MLP).

Math notes:
 - The hard-cap + redistribution step in the reference is exactly a no-op:
   dispatch after the routing floor is <= 0.4*sigmoid + 0.0375 <= 0.4375,
   while cap >= 0.5, so excess == 0 bitwise. It is therefore skipped (t unused).
 - Expert-choice top-k (k=1024 of N=8192 per (batch, expert)) is realized as
   an exact threshold: branch-free bisection on the count of logits above a
   candidate converges to theta in [v_1025, v_1024); mask = logits > theta.

On-chip layout (per core): flat tiles [128, 2048] with partition p = g*16+e
and free f = b*1024 + blk*512 + t, where token n = (blk*8+g)*512 + t.
Per-(b,e) counts are per-partition sums (stt accumulate) summed over the 8
g-groups with a single "same-e" matmul that also replicates the result.
"""

import numpy as np

B = 16
N = 8192
D = 512
H = 256
E = 16
N_CORES = 8
BPC = B // N_CORES
NT = N // 512               # 16 tiles of 512 tokens per batch
KSEL = N * 2 // E           # 1024
ALPHA = min(min(0.05, 0.15 / 4) * E, 1.0)
DSCALE = 1.0 - ALPHA        # 0.4
DFLOOR = ALPHA / E          # 0.0375
N_ITER = 30
W0 = 32.0                   # bisection range [-16, 16]; 32/2^28 ~ f32 ulp
_DEBUG = False

_prog_cache = {}


def _build(debug=False):
    import concourse.bacc as bacc
    import concourse.mybir as mybir
    import concourse.tile as tile

    F32 = mybir.dt.float32
    AO = mybir.AluOpType
    AF = mybir.ActivationFunctionType
    AX = mybir.AxisListType

    nc = bacc.Bacc("TRN2", num_devices=N_CORES)

    tokens_d = nc.dram_tensor("tokens", [BPC, N, D], F32, kind="ExternalInput")
    xyzT_d = nc.dram_tensor("xyzT", [BPC, 3, N], F32, kind="ExternalInput")
    W1_d = nc.dram_tensor("W1", [D + 3, H], F32, kind="ExternalInput")
    W2_d = nc.dram_tensor("W2", [H, E], F32, kind="ExternalInput")
    b1_d = nc.dram_tensor("b1", [H], F32, kind="ExternalInput")
    b2bc_d = nc.dram_tensor("b2bc", [128, 1], F32, kind="ExternalInput")
    crepP_d = nc.dram_tensor("crepP", [128, 3], F32, kind="ExternalInput")
    ident_d = nc.dram_tensor("ident", [128, 128], F32, kind="ExternalInput")
    selg_d = nc.dram_tensor("selg", [8, 128], F32, kind="ExternalInput")
    m2_d = nc.dram_tensor("m2", [128, 128], F32, kind="ExternalInput")

    disp_d = nc.dram_tensor("disp", [BPC, N, E], F32, kind="ExternalOutput")
    comb_d = nc.dram_tensor("comb", [BPC, N, E], F32, kind="ExternalOutput")
    if debug:
        dbg_logits_d = nc.dram_tensor("dbg_logits", [128, 2048], F32,
                                      kind="ExternalOutput")
        dbg_dists_d = nc.dram_tensor("dbg_dists", [128, 2048], F32,
                                     kind="ExternalOutput")

    with tile.TileContext(nc) as tc:
        with tc.tile_pool(name="const", bufs=1) as cpool, \
             tc.tile_pool(name="big", bufs=1) as bigpool, \
             tc.tile_pool(name="work", bufs=2) as work, \
             tc.tile_pool(name="ps", bufs=2, space="PSUM") as ps, \
             tc.tile_pool(name="dram", bufs=1, space="DRAM") as dram:

            # ---- constants ----
            w1_sb = []
            for kc in range(5):
                kch = 128 if kc < 4 else 3
                row = []
                for mc in range(2):
                    t = cpool.tile([kch, 128], F32, tag=f"w1_{kc}_{mc}",
                                   name=f"w1_{kc}_{mc}")
                    nc.sync.dma_start(
                        out=t[:], in_=W1_d[kc * 128:kc * 128 + kch,
                                           mc * 128:(mc + 1) * 128])
                    row.append(t)
                w1_sb.append(row)
            w2_sb = []
            for c in range(2):
                t = cpool.tile([128, E], F32, tag=f"w2_{c}", name=f"w2_{c}")
                nc.sync.dma_start(out=t[:], in_=W2_d[c * 128:(c + 1) * 128, :])
                w2_sb.append(t)
            b1_sb = []
            for mc in range(2):
                t = cpool.tile([128, 1], F32, tag=f"b1_{mc}", name=f"b1_{mc}")
                nc.sync.dma_start(out=t[:],
                                  in_=b1_d[mc * 128:(mc + 1) * 128].unsqueeze(1))
                b1_sb.append(t)
            b2bc_sb = cpool.tile([128, 1], F32, tag="b2bc", name="b2bc")
            nc.sync.dma_start(out=b2bc_sb[:], in_=b2bc_d[:])
            crepP_sb = cpool.tile([128, 3], F32, tag="crepP", name="crepP")
            nc.sync.dma_start(out=crepP_sb[:], in_=crepP_d[:])
            ident_sb = cpool.tile([128, 128], F32, tag="ident", name="ident")
            nc.sync.dma_start(out=ident_sb[:], in_=ident_d[:])
            selg_sb = cpool.tile([8, 128], F32, tag="selg", name="selg")
            nc.sync.dma_start(out=selg_sb[:], in_=selg_d[:])
            m2_sb = cpool.tile([128, 128], F32, tag="m2", name="m2")
            nc.sync.dma_start(out=m2_sb[:], in_=m2_d[:])
            ones_1x128 = cpool.tile([1, 128], F32, tag="o1x", name="o1x")
            nc.vector.memset(ones_1x128[:], 1.0)
            ones_128x1 = cpool.tile([128, 1], F32, tag="ox1", name="ox1")
            nc.vector.memset(ones_128x1[:], 1.0)
            ones_wide = cpool.tile([128, 1024], F32, tag="onesw", name="onesw")
            nc.vector.memset(ones_wide[:], 1.0)

            # ---- persistent tiles ----
            logits_A = bigpool.tile([128, 2048], F32, tag="logits", name="logits")
            dists_A = bigpool.tile([128, 2048], F32, tag="dists", name="dists")
            sig_A = bigpool.tile([128, 2048], F32, tag="sig", name="sig")

            # ============ Phase A: distances + global mean =================
            for b in range(BPC):
                for blk in range(2):
                    off = b * 1024 + blk * 512
                    acc = work.tile([128, 512], F32, tag="dacc", name="dacc")
                    for c in range(3):
                        xrow = work.tile([8, 512], F32, tag="xrow", name="xrow")
                        nc.sync.dma_start(
                            out=xrow[:],
                            in_=xyzT_d[b, c, blk * 4096:(blk + 1) * 4096]
                                .rearrange("(g t) -> g t", g=8))
                        p_xt = ps.tile([128, 512], F32, tag="xt", name="p_xt", bufs=1)
                        nc.tensor.matmul(p_xt[:], selg_sb[:], xrow[:],
                                         start=True, stop=True)
                        d = work.tile([128, 512], F32, tag="dtmp", name="dtmp")
                        nc.vector.tensor_scalar(out=d[:], in0=p_xt[:],
                                                scalar1=crepP_sb[:, c:c + 1],
                                                scalar2=None, op0=AO.subtract)
                        if c == 0:
                            nc.vector.tensor_tensor(out=acc[:], in0=d[:], in1=d[:],
                                                    op=AO.mult)
                        else:
                            nc.vector.tensor_tensor(out=d[:], in0=d[:], in1=d[:],
                                                    op=AO.mult)
                            nc.vector.tensor_tensor(out=acc[:], in0=acc[:], in1=d[:],
                                                    op=AO.add)
                    # sqrt + one Heron step (ACT Sqrt LUT alone is ~2e-5 rel)
                    y0 = work.tile([128, 512], F32, tag="y0", name="y0")
                    nc.scalar.activation(y0[:], acc[:], AF.Sqrt)
                    r = work.tile([128, 512], F32, tag="ry", name="ry")
                    nc.vector.reciprocal(r[:], y0[:])
                    nc.vector.tensor_tensor(out=r[:], in0=acc[:], in1=r[:], op=AO.mult)
                    nc.vector.tensor_tensor(out=r[:], in0=r[:], in1=y0[:], op=AO.add)
                    nc.vector.tensor_scalar(out=dists_A[:, off:off + 512], in0=r[:],
                                            scalar1=0.5, scalar2=None, op0=AO.mult)

            rsum = work.tile([128, 1], F32, tag="rsum", name="rsum")
            nc.vector.tensor_reduce(out=rsum[:], in_=dists_A[:], axis=AX.X, op=AO.add)
            p_tot = ps.tile([1, 1], F32, tag="xt", name="p_tot", bufs=1)
            nc.tensor.matmul(p_tot[:], ones_128x1[:], rsum[:], start=True, stop=True)
            s_tot = work.tile([1, 1], F32, tag="stot", name="stot")
            nc.vector.tensor_copy(s_tot[:], p_tot[:])
            p_bc = ps.tile([128, 1], F32, tag="xt", name="p_bc", bufs=1)
            nc.tensor.matmul(p_bc[:], ones_1x128[:], s_tot[:], start=True, stop=True)
            sb_bc = work.tile([128, 1], F32, tag="sbbc", name="sbbc")
            nc.vector.tensor_copy(sb_bc[:], p_bc[:])
            cc_in = dram.tile([128, 1], F32)
            cc_out = dram.tile([128, 1], F32, addr_space="Shared")
            nc.sync.dma_start(out=cc_in[:], in_=sb_bc[:])
            nc.gpsimd.collective_compute(
                "AllReduce", AO.add, ins=[cc_in.opt()], outs=[cc_out.opt()],
                replica_groups=[list(range(N_CORES))])
            S_sb = bigpool.tile([128, 1], F32, tag="S", name="S")
            nc.sync.dma_start(out=S_sb[:], in_=cc_out[:])
            m_sb = bigpool.tile([128, 1], F32, tag="m", name="m")
            nc.vector.tensor_scalar(out=m_sb[:], in0=S_sb[:],
                                    scalar1=1.0 / (B * N * E), scalar2=1e-6,
                                    op0=AO.mult, op1=AO.add)
            r_sb = bigpool.tile([128, 1], F32, tag="r", name="r")
            nc.vector.reciprocal(r_sb[:], m_sb[:])
            a_sb = bigpool.tile([128, 1], F32, tag="a", name="a")
            nc.vector.tensor_scalar(out=a_sb[:], in0=r_sb[:], scalar1=-1.0,
                                    scalar2=None, op0=AO.mult)

            # ---- bisect state ----
            lo, mask = [], []
            for b in range(BPC):
                lo.append(bigpool.tile([128, 1], F32, tag=f"lo{b}", name=f"lo{b}"))
                mask.append(bigpool.tile([128, 1024], F32, tag=f"mask{b}",
                                         name=f"mask{b}"))
                nc.vector.memset(lo[b][:], -W0 / 2)

            def mlp_batch(b):
                for T in range(NT):
                    blk, g = T // 8, T % 8
                    nat = []
                    for s in range(4):
                        t = work.tile([128, 512], F32, tag="nat", name="nat", bufs=6)
                        r0 = 512 * T + 128 * s
                        nc.sync.dma_start(out=t[:], in_=tokens_d[b, r0:r0 + 128, :])
                        nat.append(t)
                    tokT = []
                    for c in range(4):
                        p_t = ps.tile([128, 512], F32, tag="tokT", name="p_tokT", bufs=3)
                        for s in range(4):
                            nc.tensor.transpose(p_t[:, s * 128:(s + 1) * 128],
                                                nat[s][:, c * 128:(c + 1) * 128],
                                                ident_sb[:])
                        t_sb = work.tile([128, 512], F32, tag=f"tokT{c}",
                                         name=f"tokT{c}")
                        if c % 2 == 0:
                            nc.vector.tensor_copy(t_sb[:], p_t[:])
                        else:
                            nc.scalar.activation(t_sb[:], p_t[:], AF.Copy)
                        tokT.append(t_sb)
                    xyzw = work.tile([3, 512], F32, tag="xyzw", name="xyzw")
                    nc.sync.dma_start(out=xyzw[:],
                                      in_=xyzT_d[b, :, 512 * T:512 * (T + 1)])
                    h_sb = []
                    for mc in range(2):
                        p_h = ps.tile([128, 512], F32, tag="h", name="p_h", bufs=3)
                        for kc in range(5):
                            rhs = tokT[kc][:] if kc < 4 else xyzw[:]
                            nc.tensor.matmul(p_h[:], w1_sb[kc][mc][:], rhs,
                                             start=(kc == 0), stop=(kc == 4))
                        t_h = work.tile([128, 512], F32, tag=f"h{mc}", name=f"h{mc}")
                        nc.scalar.activation(t_h[:], p_h[:], AF.Gelu,
                                             bias=b1_sb[mc][:], scale=1.0)
                        h_sb.append(t_h)
                    p_l2 = ps.tile([16, 512], F32, tag="l2", name="p_l2", bufs=1)
                    for c in range(2):
                        nc.tensor.matmul(p_l2[:], w2_sb[c][:], h_sb[c][:],
                                         start=(c == 0), stop=(c == 1))
                    t_st = work.tile([16, 512], F32, tag="l2st", name="l2st", bufs=3)
                    nc.scalar.activation(t_st[:], p_l2[:], AF.Copy)
                    off = b * 1024 + blk * 512
                    nc.sync.dma_start(
                        out=logits_A[16 * g:16 * (g + 1), off:off + 512],
                        in_=t_st[:])

            def finalize_logits(b):
                sl = slice(b * 1024, (b + 1) * 1024)
                # logits = content + b2 + a*dists
                nc.vector.scalar_tensor_tensor(
                    out=logits_A[:, sl], in0=dists_A[:, sl], scalar=a_sb[:],
                    in1=logits_A[:, sl], op0=AO.mult, op1=AO.add)
                nc.vector.tensor_scalar(out=logits_A[:, sl], in0=logits_A[:, sl],
                                        scalar1=b2bc_sb[:], scalar2=None, op0=AO.add)
                nc.scalar.activation(sig_A[:, sl], logits_A[:, sl], AF.Sigmoid)

            def bisect(b):
                sl = slice(b * 1024, (b + 1) * 1024)
                t_mid = work.tile([128, 1], F32, tag=f"mid{b}", name=f"mid{b}",
                                  bufs=3)
                t_acc = work.tile([128, 1], F32, tag=f"pacc{b}", name=f"pacc{b}",
                                  bufs=3)
                t_s = work.tile([128, 1], F32, tag=f"sel{b}", name=f"sel{b}", bufs=3)
                for i in range(N_ITER):
                    w = W0 / (2 ** (i + 1))
                    nc.vector.tensor_scalar(out=t_mid[:], in0=lo[b][:], scalar1=w,
                                            scalar2=None, op0=AO.add)
                    nc.vector.scalar_tensor_tensor(
                        out=mask[b][:], in0=logits_A[:, sl], scalar=t_mid[:],
                        in1=ones_wide[:], op0=AO.is_gt, op1=AO.mult,
                        accum_out=t_acc[:])
                    p_cnt = ps.tile([128, 1], F32, tag="xt", name="p_cnt", bufs=1)
                    nc.tensor.matmul(p_cnt[:], m2_sb[:], t_acc[:],
                                     start=True, stop=True)
                    nc.vector.tensor_scalar(out=t_s[:], in0=p_cnt[:],
                                            scalar1=float(KSEL), scalar2=None,
                                            op0=AO.is_ge)
                    nc.vector.scalar_tensor_tensor(
                        out=lo[b][:], in0=t_s[:], scalar=w, in1=lo[b][:],
                        op0=AO.mult, op1=AO.add)

            def emit_outputs(b):
                sl = slice(b * 1024, (b + 1) * 1024)
                nc.vector.scalar_tensor_tensor(
                    out=logits_A[:, sl], in0=logits_A[:, sl], scalar=lo[b][:],
                    in1=sig_A[:, sl], op0=AO.is_gt, op1=AO.mult)
                nc.vector.tensor_scalar(out=logits_A[:, sl], in0=logits_A[:, sl],
                                        scalar1=DSCALE, scalar2=DFLOOR,
                                        op0=AO.mult, op1=AO.add)
                out_view_d = disp_d[b].rearrange(
                    "(blk g q t) e -> blk q t g e", blk=2, g=8, q=4)
                out_view_c = comb_d[b].rearrange(
                    "(blk g q t) e -> blk q t g e", blk=2, g=8, q=4)
                for blk in range(2):
                    for q in range(4):
                        off = b * 1024 + blk * 512 + q * 128
                        p_o = ps.tile([128, 128], F32, tag="h", name="p_o", bufs=3)
                        nc.tensor.transpose(p_o[:], logits_A[:, off:off + 128],
                                            ident_sb[:])
                        t_o = work.tile([128, 128], F32, tag="outT", name="outT",
                                        bufs=3)
                        nc.vector.tensor_copy(t_o[:], p_o[:])
                        t_sden = work.tile([128, 8], F32, tag="sden", name="sden",
                                           bufs=3)
                        nc.vector.tensor_reduce(
                            out=t_sden[:],
                            in_=t_o[:].rearrange("t (g e) -> t g e", g=8),
                            axis=AX.X, op=AO.add)
                        nc.vector.tensor_scalar(out=t_sden[:], in0=t_sden[:],
                                                scalar1=1e-8, scalar2=None,
                                                op0=AO.add)
                        t_rden = work.tile([128, 8], F32, tag="rden", name="rden",
                                           bufs=3)
                        nc.vector.reciprocal(t_rden[:], t_sden[:])
                        t_c = work.tile([128, 128], F32, tag="outC", name="outC",
                                        bufs=3)
                        nc.vector.tensor_tensor(
                            out=t_c[:].rearrange("t (g e) -> t g e", g=8),
                            in0=t_o[:].rearrange("t (g e) -> t g e", g=8),
                            in1=t_rden[:].unsqueeze(2).broadcast_to([128, 8, E]),
                            op=AO.mult)
                        nc.sync.dma_start(
                            out=out_view_d[blk, q],
                            in_=t_o[:].rearrange("t (g e) -> t g e", g=8))
                        nc.sync.dma_start(
                            out=out_view_c[blk, q],
                            in_=t_c[:].rearrange("t (g e) -> t g e", g=8))

            # phase order: bisect of b0 overlaps MLP of b1
            mlp_batch(0)
            finalize_logits(0)
            bisect(0)
            mlp_batch(1)
            finalize_logits(1)
            bisect(1)
            emit_outputs(0)
            emit_outputs(1)
            if debug:
                nc.sync.dma_start(out=dbg_dists_d[:], in_=dists_A[:])
                nc.sync.dma_start(out=dbg_logits_d[:], in_=logits_A[:])

    nc.finalize()
    return nc


def _get_prog(debug=False):
    key = ("prog", debug)
    if key not in _prog_cache:
        _prog_cache[key] = _build(debug)
    return _prog_cache[key]


def make_in_maps(inputs):
    tokens = np.ascontiguousarray(np.asarray(inputs["tokens"], dtype=np.float32))
    xyz = np.ascontiguousarray(np.asarray(inputs["spatial_xyz"], dtype=np.float32))
    W1 = np.ascontiguousarray(np.asarray(inputs["W1"], dtype=np.float32))
    b1 = np.asarray(inputs["b1"], dtype=np.float32)
    W2 = np.ascontiguousarray(np.asarray(inputs["W2"], dtype=np.float32))
    b2 = np.asarray(inputs["b2"], dtype=np.float32)
    centers = np.asarray(inputs["centers"], dtype=np.float32)

    b2bc = np.ascontiguousarray(np.tile(b2, 8)[:, None].astype(np.float32))
    crepP = np.ascontiguousarray(np.tile(centers, (8, 1)).astype(np.float32))
    ident = np.eye(128, dtype=np.float32)
    selg = np.ascontiguousarray(np.repeat(np.eye(8, dtype=np.float32), 16, axis=1))
    m2 = np.ascontiguousarray(
        (np.arange(128)[:, None] % 16 == np.arange(128)[None, :] % 16)
        .astype(np.float32))

    in_maps = []
    for c in range(N_CORES):
        sl = slice(BPC * c, BPC * (c + 1))
        in_maps.append({
            "tokens": tokens[sl],
            "xyzT": np.ascontiguousarray(xyz[sl].transpose(0, 2, 1)),
            "W1": W1, "W2": W2, "b1": b1,
            "b2bc": b2bc, "crepP": crepP, "ident": ident,
            "selg": selg, "m2": m2,
        })
    return in_maps


def kernel(**inputs):
    from concourse.bass_utils import run_bass_kernel_spmd

    nc = _get_prog(_DEBUG)
    in_maps = make_in_maps(inputs)
    res = run_bass_kernel_spmd(nc, in_maps, list(range(N_CORES)))
    dispatch = np.concatenate([res.results[c]["disp"] for c in range(N_CORES)], axis=0)
    combine = np.concatenate([res.results[c]["comb"] for c in range(N_CORES)], axis=0)
    if _DEBUG:
        kernel._dbg = [(res.results[c]["dbg_logits"], res.results[c]["dbg_dists"])
                       for c in range(N_CORES)]
    return dispatch, combine



# revision 10
# speedup vs baseline: 2.1397x; 2.1397x over previous
"""Trainium2 Bass kernel for nn_ModalityMoERouter (expert-choice MoE routing).

Contract: kernel(**inputs) takes the FULL inputs from reference.setup_inputs()
and returns (dispatch, combine), each (16, 8192, 16) float32.

Sharding: data-parallel over batch B=16 across 8 NeuronCores (2 batches/core);
gate weights and expert centers replicated. The global mean(dists) scalar is
computed with one AllReduce (overlapped with the gate MLP).

Key design points vs the fp32 baseline (667 us):
 - All large matmuls run in float32r (1 cycle/row on TRN2 vs 4 for fp32;
   ~11-bit mantissa, rel err ~1.5e-4 -- measured on HW; final output rel err
   ~1.4e-2, within the 2e-2 gate).
 - Tokens are transposed on the HOST (numpy) to [D, N] layout, so the kernel
   needs zero PE transposes and no PSUM-evacuation copies for activations.
 - dists(token, e) come from ONE small fp32r matmul per 512-token block using
   rows [x, y, z, |x|^2, 1] against columns [-2cx, -2cy, -2cz, 1, |c|^2];
   sqrt on ACT with accum_out gives the global-mean numerator for free.
 - Expert-choice top-k threshold via quaternary bisection (3 compares/round,
   11 rounds) using tensor_scalar is_gt + accum_out (2x_2P DVE mode).
 - Outputs are written in the on-chip (g,e)-partition layout and untransposed
   on the host.

On-chip layout: [128, 1024] per batch with partition p = g*16+e and free
f = blk*512 + t, where token n = (blk*8+g)*512 + t.
The hard-cap step of the reference is a bitwise no-op (dispatch <= 0.4375 <
cap >= 0.5), so it is skipped (t unused).
"""

import numpy as np

B = 16
N = 8192
D = 512
H = 256
E = 16
N_CORES = 8
BPC = B // N_CORES
KSEL = N * 2 // E           # 1024
ALPHA = min(min(0.05, 0.15 / 4) * E, 1.0)
DSCALE = 1.0 - ALPHA        # 0.4
DFLOOR = ALPHA / E          # 0.0375
N_ROUNDS = 11               # quaternary bisection rounds: window 32/4^11
_DEBUG = False

_prog_cache = {}


def _build(debug=False):
    import concourse.bacc as bacc
    import concourse.mybir as mybir
    import concourse.tile as tile

    F32 = mybir.dt.float32
    F32R = mybir.dt.float32r
    AO = mybir.AluOpType
    AF = mybir.ActivationFunctionType

    nc = bacc.Bacc("TRN2", num_devices=N_CORES)

    tokT_d = nc.dram_tensor("tokT", [BPC, 515, N], F32R, kind="ExternalInput")
    xp_d = nc.dram_tensor("xp", [40, 2048], F32R, kind="ExternalInput")
    C5_d = nc.dram_tensor("C5", [40, 128], F32R, kind="ExternalInput")
    W1p_d = nc.dram_tensor("W1p", [515, 256], F32R, kind="ExternalInput")
    W2p_d = nc.dram_tensor("W2p", [128, 32], F32R, kind="ExternalInput")
    b1p_d = nc.dram_tensor("b1p", [128, 2], F32, kind="ExternalInput")
    b2bc_d = nc.dram_tensor("b2bc", [128, 1], F32, kind="ExternalInput")
    m2_d = nc.dram_tensor("m2", [128, 128], F32, kind="ExternalInput")
    sel8_d = nc.dram_tensor("sel8", [128, 8], F32R, kind="ExternalInput")
    rep8_d = nc.dram_tensor("rep8", [8, 128], F32R, kind="ExternalInput")

    disp_d = nc.dram_tensor("disp", [BPC, 128, 1024], F32, kind="ExternalOutput")
    comb_d = nc.dram_tensor("comb", [BPC, 128, 1024], F32, kind="ExternalOutput")
    if debug:
        dbg_logits_d = nc.dram_tensor("dbg_logits", [128, 2048], F32,
                                      kind="ExternalOutput")
        dbg_dists_d = nc.dram_tensor("dbg_dists", [128, 2048], F32,
                                     kind="ExternalOutput")

    with tile.TileContext(nc) as tc:
        with tc.tile_pool(name="const", bufs=1) as cpool, \
             tc.tile_pool(name="big", bufs=1) as bigpool, \
             tc.tile_pool(name="tok", bufs=2) as tokp, \
             tc.tile_pool(name="hb", bufs=4) as hb, \
             tc.tile_pool(name="work", bufs=2) as work, \
             tc.tile_pool(name="ps", bufs=1, space="PSUM") as ps, \
             tc.tile_pool(name="dram", bufs=1, space="DRAM") as dram:

            # ---- constants ----
            w1_sb = []
            for kc in range(5):
                kch = 128 if kc < 4 else 3
                t = cpool.tile([kch, 256], F32R, tag=f"w1_{kc}", name=f"w1_{kc}")
                nc.sync.dma_start(out=t[:], in_=W1p_d[kc * 128:kc * 128 + kch, :])
                w1_sb.append(t)
            w2_sb = cpool.tile([128, 32], F32R, tag="w2", name="w2")
            nc.sync.dma_start(out=w2_sb[:], in_=W2p_d[:])
            b1_sb = cpool.tile([128, 2], F32, tag="b1", name="b1")
            nc.sync.dma_start(out=b1_sb[:], in_=b1p_d[:])
            b2bc_sb = cpool.tile([128, 1], F32, tag="b2bc", name="b2bc")
            nc.sync.dma_start(out=b2bc_sb[:], in_=b2bc_d[:])
            C5_sb = cpool.tile([40, 128], F32R, tag="C5", name="C5")
            nc.sync.dma_start(out=C5_sb[:], in_=C5_d[:])
            xp_sb = cpool.tile([40, 2048], F32R, tag="xp", name="xp")
            nc.sync.dma_start(out=xp_sb[:], in_=xp_d[:])
            m2_sb = cpool.tile([128, 128], F32, tag="m2", name="m2")
            nc.sync.dma_start(out=m2_sb[:], in_=m2_d[:])
            sel8_sb = cpool.tile([128, 8], F32R, tag="sel8", name="sel8")
            nc.sync.dma_start(out=sel8_sb[:], in_=sel8_d[:])
            rep8_sb = cpool.tile([8, 128], F32R, tag="rep8", name="rep8")
            nc.sync.dma_start(out=rep8_sb[:], in_=rep8_d[:])
            ones_1x128 = cpool.tile([1, 128], F32, tag="o1x", name="o1x")
            nc.vector.memset(ones_1x128[:], 1.0)
            ones_128x1 = cpool.tile([128, 1], F32, tag="ox1", name="ox1")
            nc.vector.memset(ones_128x1[:], 1.0)

            # ---- persistent tiles ----
            logits_A = bigpool.tile([128, 2048], F32, tag="logits", name="logits")
            dists_A = bigpool.tile([128, 2048], F32, tag="dists", name="dists")
            sig_A = bigpool.tile([128, 2048], F32, tag="sig", name="sig")

            # ============ dists + global mean (both batches early) ==========
            racc = bigpool.tile([128, 4], F32, tag="racc", name="racc")
            for b in range(BPC):
                for blk in range(2):
                    i = 2 * b + blk
                    p_d = ps.tile([128, 512], F32, tag="ph", name="p_d", bufs=3)
                    nc.tensor.matmul(p_d[:], C5_sb[:],
                                     xp_sb[:, i * 512:(i + 1) * 512],
                                     start=True, stop=True)
                    off = b * 1024 + blk * 512
                    nc.scalar.activation(dists_A[:, off:off + 512], p_d[:],
                                         AF.Sqrt, accum_out=racc[:, i:i + 1])

            rsum = work.tile([128, 1], F32, tag="rsum", name="rsum")
            nc.vector.tensor_tensor(out=rsum[:], in0=racc[:, 0:1],
                                    in1=racc[:, 1:2], op=AO.add)
            nc.vector.tensor_tensor(out=rsum[:], in0=rsum[:],
                                    in1=racc[:, 2:3], op=AO.add)
            nc.vector.tensor_tensor(out=rsum[:], in0=rsum[:],
                                    in1=racc[:, 3:4], op=AO.add)
            p_tot = ps.tile([1, 1], F32, tag="pcnt", name="p_tot", bufs=1)
            nc.tensor.matmul(p_tot[:], ones_128x1[:], rsum[:], start=True, stop=True)
            s_tot = work.tile([1, 1], F32, tag="stot", name="stot")
            nc.vector.tensor_copy(s_tot[:], p_tot[:])
            p_bc = ps.tile([128, 1], F32, tag="pcnt", name="p_bc", bufs=1)
            nc.tensor.matmul(p_bc[:], ones_1x128[:], s_tot[:], start=True, stop=True)
            sb_bc = work.tile([128, 1], F32, tag="sbbc", name="sbbc")
            nc.vector.tensor_copy(sb_bc[:], p_bc[:])
            cc_in = dram.tile([128, 1], F32)
            cc_out = dram.tile([128, 1], F32, addr_space="Shared")
            nc.sync.dma_start(out=cc_in[:], in_=sb_bc[:])
            nc.gpsimd.collective_compute(
                "AllReduce", AO.add, ins=[cc_in.opt()], outs=[cc_out.opt()],
                replica_groups=[list(range(N_CORES))])
            S_sb = bigpool.tile([128, 1], F32, tag="S", name="S")
            nc.sync.dma_start(out=S_sb[:], in_=cc_out[:])
            m_sb = bigpool.tile([128, 1], F32, tag="m", name="m")
            nc.vector.tensor_scalar(out=m_sb[:], in0=S_sb[:],
                                    scalar1=1.0 / (B * N * E), scalar2=1e-6,
                                    op0=AO.mult, op1=AO.add)
            r_sb = bigpool.tile([128, 1], F32, tag="r", name="r")
            nc.vector.reciprocal(r_sb[:], m_sb[:])
            a_sb = bigpool.tile([128, 1], F32, tag="a", name="a")
            nc.vector.tensor_scalar(out=a_sb[:], in0=r_sb[:], scalar1=-1.0,
                                    scalar2=None, op0=AO.mult)

            # ============ gate MLP =========================================
            def mlp_batch(b):
                # eighths of 1024 tokens; one merged DMA for the 4 D-chunks
                for q in range(8):
                    tk = tokp.tile([128, 4096], F32R, tag="tk", name="tk")
                    nc.sync.dma_start(
                        out=tk[:].rearrange("p (kc c) -> p kc c", kc=4),
                        in_=tokT_d[b, 0:512, q * 1024:(q + 1) * 1024]
                        .rearrange("(kc p) c -> p kc c", kc=4))
                    tx = tokp.tile([3, 1024], F32R, tag="tx", name="tx")
                    nc.sync.dma_start(
                        out=tx[:],
                        in_=tokT_d[b, 512:515, q * 1024:(q + 1) * 1024])
                    for s in range(2):
                        T = 2 * q + s            # chunk id: blk = T//8, g = T%8
                        h_sb = []
                        for mc in range(2):
                            p_h = ps.tile([128, 512], F32, tag="ph",
                                          name="p_h", bufs=3)
                            for kc in range(5):
                                rhs = (tk[:, kc * 1024 + s * 512:
                                          kc * 1024 + (s + 1) * 512]
                                       if kc < 4 else
                                       tx[:, s * 512:(s + 1) * 512])
                                nc.tensor.matmul(
                                    p_h[:],
                                    w1_sb[kc][:, mc * 128:(mc + 1) * 128],
                                    rhs,
                                    start=(kc == 0), stop=(kc == 4))
                            t_h = hb.tile([128, 512], F32R, tag=f"h{mc}",
                                          name=f"h{mc}")
                            nc.scalar.activation(t_h[:], p_h[:], AF.Gelu,
                                                 bias=b1_sb[:, mc:mc + 1],
                                                 scale=1.0)
                            h_sb.append(t_h)
                        p_l2 = ps.tile([16, 512], F32, tag="pl2",
                                       name="p_l2", bufs=2)
                        for mc in range(2):
                            nc.tensor.matmul(p_l2[:],
                                             w2_sb[:, mc * 16:(mc + 1) * 16],
                                             h_sb[mc][:],
                                             start=(mc == 0), stop=(mc == 1))
                        t_st = hb.tile([16, 512], F32, tag="tst", name="tst",
                                       bufs=3)
                        nc.scalar.activation(t_st[:], p_l2[:], AF.Copy)
                        blk, g = T // 8, T % 8
                        nc.sync.dma_start(
                            out=logits_A[16 * g:16 * (g + 1),
                                         b * 1024 + blk * 512:
                                         b * 1024 + blk * 512 + 512],
                            in_=t_st[:])

            def finalize(b):
                sl = slice(b * 1024, (b + 1) * 1024)
                nc.vector.scalar_tensor_tensor(
                    out=logits_A[:, sl], in0=dists_A[:, sl], scalar=a_sb[:],
                    in1=logits_A[:, sl], op0=AO.mult, op1=AO.add)
                nc.vector.tensor_scalar(out=logits_A[:, sl], in0=logits_A[:, sl],
                                        scalar1=b2bc_sb[:], scalar2=None,
                                        op0=AO.add)
                nc.scalar.activation(sig_A[:, sl], logits_A[:, sl], AF.Sigmoid)

            # quaternary bisection for the top-KSEL threshold per (b,e)
            theta = [bigpool.tile([128, 1], F32, tag=f"th{b}", name=f"th{b}")
                     for b in range(BPC)]

            def bisect(b):
                sl = slice(b * 1024, (b + 1) * 1024)
                mu = theta[b]
                nc.vector.memset(mu[:], 0.0)
                msk = work.tile([128, 1024], F32, tag=f"msk{b}",
                                name=f"msk{b}", bufs=1)
                cj = work.tile([128, 3], F32, tag=f"cj{b}", name=f"cj{b}",
                               bufs=2)
                acc = work.tile([128, 3], F32, tag=f"ac{b}", name=f"ac{b}",
                                bufs=2)
                s3 = work.tile([128, 3], F32, tag=f"s3{b}", name=f"s3{b}",
                               bufs=2)
                sig1 = work.tile([128, 1], F32, tag=f"sg{b}", name=f"sg{b}",
                                 bufs=2)
                K = 0.0
                Wd = 32.0
                for r in range(N_ROUNDS):
                    for j in range(3):
                        nc.vector.tensor_scalar(
                            out=cj[:, j:j + 1], in0=mu[:],
                            scalar1=K + (j - 1) * Wd / 4, scalar2=None,
                            op0=AO.add)
                        nc.vector.tensor_scalar(
                            out=msk[:], in0=logits_A[:, sl],
                            scalar1=cj[:, j:j + 1], scalar2=0.0,
                            op0=AO.is_gt, op1=AO.add,
                            accum_out=acc[:, j:j + 1])
                    p_cnt = ps.tile([128, 3], F32, tag="pcnt", name="p_cnt",
                                    bufs=1)
                    nc.tensor.matmul(p_cnt[:], m2_sb[:], acc[:],
                                     start=True, stop=True)
                    nc.vector.tensor_scalar(
                        out=s3[:], in0=p_cnt[:], scalar1=float(KSEL),
                        scalar2=0.0, op0=AO.is_ge, op1=AO.add,
                        accum_out=sig1[:])
                    nc.vector.scalar_tensor_tensor(
                        out=mu[:], in0=sig1[:], scalar=Wd / 4, in1=mu[:],
                        op0=AO.mult, op1=AO.add)
                    K = K - 0.375 * Wd
                    Wd = Wd / 4
                # theta := left edge of the final interval
                nc.vector.tensor_scalar(out=mu[:], in0=mu[:],
                                        scalar1=K - Wd / 2, scalar2=None,
                                        op0=AO.add)

            def emit(b):
                sl = slice(b * 1024, (b + 1) * 1024)
                # reuse the bisect mask buffer as dispM scratch
                dispM = work.tile([128, 1024], F32, tag=f"msk{b}",
                                  name=f"dm{b}", bufs=1)
                nc.vector.scalar_tensor_tensor(
                    out=dispM[:], in0=logits_A[:, sl], scalar=theta[b][:],
                    in1=sig_A[:, sl], op0=AO.is_gt, op1=AO.mult)
                dispO = work.tile([128, 1024], F32R, tag=f"do{b}",
                                  name=f"do{b}", bufs=1)
                nc.vector.tensor_scalar(out=dispO[:], in0=dispM[:],
                                        scalar1=DSCALE, scalar2=DFLOOR,
                                        op0=AO.mult, op1=AO.add)
                nc.sync.dma_start(out=disp_d[b], in_=dispO[:].bitcast(F32))
                # combine = disp / rowsum_e(disp): compact reciprocal
                dc = work.tile([8, 1024], F32R, tag=f"dc{b}", name=f"dc{b}",
                               bufs=1)
                for hhalf in range(2):
                    p_dc = ps.tile([8, 512], F32, tag="pdc", name="p_dc",
                                   bufs=2)
                    nc.tensor.matmul(p_dc[:], sel8_sb[:],
                                     dispO[:, hhalf * 512:(hhalf + 1) * 512],
                                     start=True, stop=True)
                    nc.scalar.activation(dc[:, hhalf * 512:(hhalf + 1) * 512],
                                         p_dc[:], AF.Copy)
                rcmp = work.tile([128, 64], F32R, tag=f"rc{b}", name=f"rc{b}",
                                 bufs=1)
                nc.sync.dma_start(
                    out=rcmp[:],
                    in_=dc[:].rearrange("g (q c) -> (g q) c", q=16))
                with nc.allow_low_precision(reason="combine denom recip to "
                                            "f32r; 1.2e-4 rel, tol 2e-2"):
                    nc.vector.reciprocal(rcmp[:], rcmp[:])
                rden8 = work.tile([8, 1024], F32R, tag=f"dc{b}", name=f"rd{b}",
                                  bufs=1)
                nc.sync.dma_start(
                    out=rden8[:].rearrange("g (q c) -> (g q) c", q=16),
                    in_=rcmp[:])
                comb = work.tile([128, 1024], F32, tag=f"cb{b}", name=f"cb{b}",
                                 bufs=1)
                for hhalf in range(2):
                    hsl = slice(hhalf * 512, (hhalf + 1) * 512)
                    p_rep = ps.tile([128, 512], F32, tag="ph", name="p_rep",
                                    bufs=3)
                    nc.tensor.matmul(p_rep[:], rep8_sb[:], rden8[:, hsl],
                                     start=True, stop=True)
                    nc.vector.tensor_tensor(out=comb[:, hsl],
                                            in0=dispO[:, hsl].bitcast(F32),
                                            in1=p_rep[:], op=AO.mult)
                nc.sync.dma_start(out=comb_d[b], in_=comb[:])

            mlp_batch(0)
            mlp_batch(1)
            finalize(0)
            bisect(0)
            emit(0)
            finalize(1)
            bisect(1)
            emit(1)
            if debug:
                nc.sync.dma_start(out=dbg_dists_d[:], in_=dists_A[:])
                nc.sync.dma_start(out=dbg_logits_d[:], in_=logits_A[:])

    nc.finalize()
    return nc


def _get_prog(debug=False):
    key = ("prog", debug)
    if key not in _prog_cache:
        _prog_cache[key] = _build(debug)
    return _prog_cache[key]


def make_in_maps(inputs):
    tokens = np.asarray(inputs["tokens"], dtype=np.float32)
    xyz = np.asarray(inputs["spatial_xyz"], dtype=np.float32)
    W1 = np.ascontiguousarray(np.asarray(inputs["W1"], dtype=np.float32))
    b1 = np.asarray(inputs["b1"], dtype=np.float32)
    W2 = np.asarray(inputs["W2"], dtype=np.float32)
    b2 = np.asarray(inputs["b2"], dtype=np.float32)
    centers = np.asarray(inputs["centers"], dtype=np.float32)

    # shared constants
    W2p = np.zeros((128, 32), np.float32)
    for mc in range(2):
        W2p[:, mc * 16:(mc + 1) * 16] = W2[mc * 128:(mc + 1) * 128, :]
    b1p = np.ascontiguousarray(b1.reshape(2, 128).T)
    b2bc = np.ascontiguousarray(np.tile(b2, 8)[:, None].astype(np.float32))
    m2 = np.ascontiguousarray(
        (np.arange(128)[:, None] % 16 == np.arange(128)[None, :] % 16)
        .astype(np.float32))
    sel8 = np.ascontiguousarray(
        (np.arange(128)[:, None] // 16 == np.arange(8)[None, :])
        .astype(np.float32))
    rep8 = np.ascontiguousarray(sel8.T)
    # C5 [40, 128]: col 16g+e, rows 5g+c = [-2cx, -2cy, -2cz, 1, |c|^2]
    C5 = np.zeros((40, 128), np.float32)
    csq = (centers.astype(np.float64) ** 2).sum(-1).astype(np.float32)
    for g in range(8):
        for e in range(E):
            p = 16 * g + e
            C5[5 * g + 0, p] = -2.0 * centers[e, 0]
            C5[5 * g + 1, p] = -2.0 * centers[e, 1]
            C5[5 * g + 2, p] = -2.0 * centers[e, 2]
            C5[5 * g + 3, p] = 1.0
            C5[5 * g + 4, p] = csq[e]

    in_maps = []
    for c in range(N_CORES):
        tokT = np.empty((BPC, 515, N), np.float32)
        xp = np.empty((40, 2048), np.float32)
        for b in range(BPC):
            gb = BPC * c + b
            tokT[b, :512] = tokens[gb].T
            tokT[b, 512:515] = xyz[gb].T
            x2 = (xyz[gb].astype(np.float64) ** 2).sum(-1).astype(np.float32)
            for blk in range(2):
                i = 2 * b + blk
                # rows 5g+c over tokens (g, t) of this blk
                xs = xyz[gb][(blk * 8) * 512:(blk * 8 + 8) * 512]  # (8*512, 3)
                xs = xs.reshape(8, 512, 3)
                x2s = x2[(blk * 8) * 512:(blk * 8 + 8) * 512].reshape(8, 512)
                blkm = np.empty((8, 5, 512), np.float32)
                blkm[:, 0:3] = xs.transpose(0, 2, 1)
                blkm[:, 3] = x2s
                blkm[:, 4] = 1.0
                xp[:, i * 512:(i + 1) * 512] = blkm.reshape(40, 512)
        in_maps.append({
            "tokT": np.ascontiguousarray(tokT),
            "xp": np.ascontiguousarray(xp),
            "C5": C5, "W1p": W1, "W2p": W2p, "b1p": b1p, "b2bc": b2bc,
            "m2": m2, "sel8": sel8, "rep8": rep8,
        })
    return in_maps


def _unlayout(arr):
    """[128, 1024] (p=16g+e, f=blk*512+t) -> (8192, 16)"""
    return (arr.reshape(8, 16, 2, 512).transpose(2, 0, 3, 1)
            .reshape(N, E))


def kernel(**inputs):
    from concourse.bass_utils import run_bass_kernel_spmd

    nc = _get_prog(_DEBUG)
    in_maps = make_in_maps(inputs)
    res = run_bass_kernel_spmd(nc, in_maps, list(range(N_CORES)))
    dispatch = np.empty((B, N, E), np.float32)
    combine = np.empty((B, N, E), np.float32)
    for c in range(N_CORES):
        for b in range(BPC):
            dispatch[BPC * c + b] = _unlayout(res.results[c]["disp"][b])
            combine[BPC * c + b] = _unlayout(res.results[c]["comb"][b])
    if _DEBUG:
        kernel._dbg = [(res.results[c]["dbg_logits"], res.results[c]["dbg_dists"])
                       for c in range(N_CORES)]
    return dispatch, combine


# revision 16
# speedup vs baseline: 2.1583x; 1.0087x over previous
"""Trainium2 Bass kernel for nn_ModalityMoERouter (expert-choice MoE routing).

Contract: kernel(**inputs) takes the FULL inputs from reference.setup_inputs()
and returns (dispatch, combine), each (16, 8192, 16) float32.

Sharding: data-parallel over batch B=16 across 8 NeuronCores (2 batches/core);
gate weights and expert centers replicated. The global mean(dists) scalar is
computed with one AllReduce (overlapped with the gate MLP).

Key design points vs the fp32 baseline (667 us):
 - All large matmuls run in float32r (1 cycle/row on TRN2 vs 4 for fp32;
   ~11-bit mantissa, rel err ~1.5e-4 -- measured on HW; final output rel err
   ~1.4e-2, within the 2e-2 gate).
 - Tokens are transposed on the HOST (numpy) to [D, N] layout, so the kernel
   needs zero PE transposes and no PSUM-evacuation copies for activations.
 - dists(token, e) come from ONE small fp32r matmul per 512-token block using
   rows [x, y, z, |x|^2, 1] against columns [-2cx, -2cy, -2cz, 1, |c|^2];
   sqrt on ACT with accum_out gives the global-mean numerator for free.
 - Expert-choice top-k threshold via quaternary bisection (3 compares/round,
   11 rounds) using tensor_scalar is_gt + accum_out (2x_2P DVE mode).
 - Outputs are written in the on-chip (g,e)-partition layout and untransposed
   on the host.

On-chip layout: [128, 1024] per batch with partition p = g*16+e and free
f = blk*512 + t, where token n = (blk*8+g)*512 + t.
The hard-cap step of the reference is a bitwise no-op (dispatch <= 0.4375 <
cap >= 0.5), so it is skipped (t unused).
"""

import numpy as np

B = 16
N = 8192
D = 512
H = 256
E = 16
N_CORES = 8
BPC = B // N_CORES
KSEL = N * 2 // E           # 1024
ALPHA = min(min(0.05, 0.15 / 4) * E, 1.0)
DSCALE = 1.0 - ALPHA        # 0.4
DFLOOR = ALPHA / E          # 0.0375
N_ROUNDS = 11               # quaternary bisection rounds: window 32/4^11
_DEBUG = False

_prog_cache = {}


def _build(debug=False):
    import concourse.bacc as bacc
    import concourse.mybir as mybir
    import concourse.tile as tile

    F32 = mybir.dt.float32
    F32R = mybir.dt.float32r
    AO = mybir.AluOpType
    AF = mybir.ActivationFunctionType

    nc = bacc.Bacc("TRN2", num_devices=N_CORES)

    tokT_d = nc.dram_tensor("tokT", [BPC, 515, N], F32R, kind="ExternalInput")
    xp_d = nc.dram_tensor("xp", [40, 2048], F32R, kind="ExternalInput")
    C5_d = nc.dram_tensor("C5", [40, 128], F32R, kind="ExternalInput")
    W1p_d = nc.dram_tensor("W1p", [515, 256], F32R, kind="ExternalInput")
    W2p_d = nc.dram_tensor("W2p", [128, 32], F32R, kind="ExternalInput")
    b1p_d = nc.dram_tensor("b1p", [128, 2], F32, kind="ExternalInput")
    b2bc_d = nc.dram_tensor("b2bc", [128, 1], F32, kind="ExternalInput")
    m2_d = nc.dram_tensor("m2", [128, 128], F32, kind="ExternalInput")
    mrep_d = nc.dram_tensor("mrep", [128, 128], F32R, kind="ExternalInput")

    disp_d = nc.dram_tensor("disp", [BPC, 128, 1024], F32, kind="ExternalOutput")
    comb_d = nc.dram_tensor("comb", [BPC, 128, 1024], F32, kind="ExternalOutput")
    if debug:
        dbg_logits_d = nc.dram_tensor("dbg_logits", [128, 2048], F32,
                                      kind="ExternalOutput")
        dbg_dists_d = nc.dram_tensor("dbg_dists", [128, 2048], F32,
                                     kind="ExternalOutput")

    with tile.TileContext(nc) as tc:
        with tc.tile_pool(name="const", bufs=1) as cpool, \
             tc.tile_pool(name="big", bufs=1) as bigpool, \
             tc.tile_pool(name="tok", bufs=2) as tokp, \
             tc.tile_pool(name="hb", bufs=4) as hb, \
             tc.tile_pool(name="work", bufs=2) as work, \
             tc.tile_pool(name="ps", bufs=1, space="PSUM") as ps, \
             tc.tile_pool(name="dram", bufs=1, space="DRAM") as dram:

            # ---- constants ----
            w1_sb = []
            for kc in range(5):
                kch = 128 if kc < 4 else 3
                t = cpool.tile([kch, 256], F32R, tag=f"w1_{kc}", name=f"w1_{kc}")
                nc.sync.dma_start(out=t[:], in_=W1p_d[kc * 128:kc * 128 + kch, :])
                w1_sb.append(t)
            w2_sb = cpool.tile([128, 32], F32R, tag="w2", name="w2")
            nc.sync.dma_start(out=w2_sb[:], in_=W2p_d[:])
            b1_sb = cpool.tile([128, 2], F32, tag="b1", name="b1")
            nc.sync.dma_start(out=b1_sb[:], in_=b1p_d[:])
            b2bc_sb = cpool.tile([128, 1], F32, tag="b2bc", name="b2bc")
            nc.sync.dma_start(out=b2bc_sb[:], in_=b2bc_d[:])
            C5_sb = cpool.tile([40, 128], F32R, tag="C5", name="C5")
            nc.sync.dma_start(out=C5_sb[:], in_=C5_d[:])
            xp_sb = cpool.tile([40, 2048], F32R, tag="xp", name="xp")
            nc.sync.dma_start(out=xp_sb[:], in_=xp_d[:])
            m2_sb = cpool.tile([128, 128], F32, tag="m2", name="m2")
            nc.sync.dma_start(out=m2_sb[:], in_=m2_d[:])
            mrep_sb = cpool.tile([128, 128], F32R, tag="mrep", name="mrep")
            nc.sync.dma_start(out=mrep_sb[:], in_=mrep_d[:])
            ones_1x128 = cpool.tile([1, 128], F32, tag="o1x", name="o1x")
            nc.vector.memset(ones_1x128[:], 1.0)
            ones_128x1 = cpool.tile([128, 1], F32, tag="ox1", name="ox1")
            nc.vector.memset(ones_128x1[:], 1.0)

            # ---- persistent tiles ----
            logits_A = bigpool.tile([128, 2048], F32, tag="logits", name="logits")
            dists_A = bigpool.tile([128, 2048], F32, tag="dists", name="dists")
            sig_A = bigpool.tile([128, 2048], F32, tag="sig", name="sig")

            # ============ dists + global mean (both batches early) ==========
            for b in range(BPC):
                for blk in range(2):
                    i = 2 * b + blk
                    p_d = ps.tile([128, 512], F32, tag="ph", name="p_d", bufs=3)
                    nc.tensor.matmul(p_d[:], C5_sb[:],
                                     xp_sb[:, i * 512:(i + 1) * 512],
                                     start=True, stop=True)
                    off = b * 1024 + blk * 512
                    nc.scalar.activation(dists_A[:, off:off + 512], p_d[:],
                                         AF.Sqrt)

            rsum = work.tile([128, 1], F32, tag="rsum", name="rsum")
            nc.vector.tensor_reduce(out=rsum[:], in_=dists_A[:],
                                    axis=mybir.AxisListType.X, op=AO.add)
            p_tot = ps.tile([1, 1], F32, tag="pcnt", name="p_tot", bufs=1)
            nc.tensor.matmul(p_tot[:], ones_128x1[:], rsum[:], start=True, stop=True)
            s_tot = work.tile([1, 1], F32, tag="stot", name="stot")
            nc.vector.tensor_copy(s_tot[:], p_tot[:])
            p_bc = ps.tile([128, 1], F32, tag="pcnt", name="p_bc", bufs=1)
            nc.tensor.matmul(p_bc[:], ones_1x128[:], s_tot[:], start=True, stop=True)
            sb_bc = work.tile([128, 1], F32, tag="sbbc", name="sbbc")
            nc.vector.tensor_copy(sb_bc[:], p_bc[:])
            cc_in = dram.tile([128, 1], F32)
            cc_out = dram.tile([128, 1], F32, addr_space="Shared")
            nc.sync.dma_start(out=cc_in[:], in_=sb_bc[:])
            nc.gpsimd.collective_compute(
                "AllReduce", AO.add, ins=[cc_in.opt()], outs=[cc_out.opt()],
                replica_groups=[list(range(N_CORES))])
            S_sb = bigpool.tile([128, 1], F32, tag="S", name="S")
            nc.sync.dma_start(out=S_sb[:], in_=cc_out[:])
            m_sb = bigpool.tile([128, 1], F32, tag="m", name="m")
            nc.vector.tensor_scalar(out=m_sb[:], in0=S_sb[:],
                                    scalar1=1.0 / (B * N * E), scalar2=1e-6,
                                    op0=AO.mult, op1=AO.add)
            r_sb = bigpool.tile([128, 1], F32, tag="r", name="r")
            nc.vector.reciprocal(r_sb[:], m_sb[:])
            a_sb = bigpool.tile([128, 1], F32, tag="a", name="a")
            nc.vector.tensor_scalar(out=a_sb[:], in0=r_sb[:], scalar1=-1.0,
                                    scalar2=None, op0=AO.mult)

            # ============ gate MLP =========================================
            def mlp_batch(b):
                # eighths of 1024 tokens; one merged DMA for the 4 D-chunks
                for q in range(8):
                    tk = tokp.tile([128, 4096], F32R, tag="tk", name="tk")
                    nc.sync.dma_start(
                        out=tk[:].rearrange("p (kc c) -> p kc c", kc=4),
                        in_=tokT_d[b, 0:512, q * 1024:(q + 1) * 1024]
                        .rearrange("(kc p) c -> p kc c", kc=4))
                    tx = tokp.tile([3, 1024], F32R, tag="tx", name="tx")
                    nc.sync.dma_start(
                        out=tx[:],
                        in_=tokT_d[b, 512:515, q * 1024:(q + 1) * 1024])
                    for s in range(2):
                        T = 2 * q + s            # chunk id: blk = T//8, g = T%8
                        h_sb = []
                        for mc in range(2):
                            p_h = ps.tile([128, 512], F32, tag="ph",
                                          name="p_h", bufs=3)
                            for kc in range(5):
                                rhs = (tk[:, kc * 1024 + s * 512:
                                          kc * 1024 + (s + 1) * 512]
                                       if kc < 4 else
                                       tx[:, s * 512:(s + 1) * 512])
                                nc.tensor.matmul(
                                    p_h[:],
                                    w1_sb[kc][:, mc * 128:(mc + 1) * 128],
                                    rhs,
                                    start=(kc == 0), stop=(kc == 4))
                            t_h = hb.tile([128, 512], F32R, tag=f"h{mc}",
                                          name=f"h{mc}")
                            nc.scalar.activation(t_h[:], p_h[:], AF.Gelu,
                                                 bias=b1_sb[:, mc:mc + 1],
                                                 scale=1.0)
                            h_sb.append(t_h)
                        p_l2 = ps.tile([16, 512], F32, tag="pl2",
                                       name="p_l2", bufs=2)
                        for mc in range(2):
                            nc.tensor.matmul(p_l2[:],
                                             w2_sb[:, mc * 16:(mc + 1) * 16],
                                             h_sb[mc][:],
                                             start=(mc == 0), stop=(mc == 1))
                        t_st = hb.tile([16, 512], F32, tag="tst", name="tst",
                                       bufs=3)
                        nc.scalar.activation(t_st[:], p_l2[:], AF.Copy)
                        blk, g = T // 8, T % 8
                        nc.sync.dma_start(
                            out=logits_A[16 * g:16 * (g + 1),
                                         b * 1024 + blk * 512:
                                         b * 1024 + blk * 512 + 512],
                            in_=t_st[:])

            def finalize(b):
                sl = slice(b * 1024, (b + 1) * 1024)
                nc.vector.scalar_tensor_tensor(
                    out=logits_A[:, sl], in0=dists_A[:, sl], scalar=a_sb[:],
                    in1=logits_A[:, sl], op0=AO.mult, op1=AO.add)
                nc.vector.tensor_scalar(out=logits_A[:, sl], in0=logits_A[:, sl],
                                        scalar1=b2bc_sb[:], scalar2=None,
                                        op0=AO.add)
                nc.scalar.activation(sig_A[:, sl], logits_A[:, sl], AF.Sigmoid)

            # quaternary bisection for the top-KSEL threshold per (b,e)
            theta = [bigpool.tile([128, 1], F32, tag=f"th{b}", name=f"th{b}")
                     for b in range(BPC)]

            def bisect(b):
                sl = slice(b * 1024, (b + 1) * 1024)
                mu = theta[b]
                nc.vector.memset(mu[:], 0.0)
                msk = work.tile([128, 1024], F32, tag=f"msk{b}",
                                name=f"msk{b}", bufs=1)
                cj = work.tile([128, 3], F32, tag=f"cj{b}", name=f"cj{b}",
                               bufs=2)
                acc = work.tile([128, 3], F32, tag=f"ac{b}", name=f"ac{b}",
                                bufs=2)
                s3 = work.tile([128, 3], F32, tag=f"s3{b}", name=f"s3{b}",
                               bufs=2)
                sig1 = work.tile([128, 1], F32, tag=f"sg{b}", name=f"sg{b}",
                                 bufs=2)
                K = 0.0
                Wd = 32.0
                for r in range(N_ROUNDS):
                    for j in range(3):
                        nc.vector.tensor_scalar(
                            out=cj[:, j:j + 1], in0=mu[:],
                            scalar1=K + (j - 1) * Wd / 4, scalar2=None,
                            op0=AO.add)
                        nc.vector.tensor_scalar(
                            out=msk[:], in0=logits_A[:, sl],
                            scalar1=cj[:, j:j + 1], scalar2=0.0,
                            op0=AO.is_gt, op1=AO.add,
                            accum_out=acc[:, j:j + 1])
                    p_cnt = ps.tile([128, 3], F32, tag="pcnt", name="p_cnt",
                                    bufs=1)
                    nc.tensor.matmul(p_cnt[:], m2_sb[:], acc[:],
                                     start=True, stop=True)
                    nc.vector.tensor_scalar(
                        out=s3[:], in0=p_cnt[:], scalar1=float(KSEL),
                        scalar2=0.0, op0=AO.is_ge, op1=AO.add,
                        accum_out=sig1[:])
                    nc.vector.scalar_tensor_tensor(
                        out=mu[:], in0=sig1[:], scalar=Wd / 4, in1=mu[:],
                        op0=AO.mult, op1=AO.add)
                    K = K - 0.375 * Wd
                    Wd = Wd / 4
                # theta := left edge of the final interval
                nc.vector.tensor_scalar(out=mu[:], in0=mu[:],
                                        scalar1=K - Wd / 2, scalar2=None,
                                        op0=AO.add)

            def emit(b):
                sl = slice(b * 1024, (b + 1) * 1024)
                # reuse the bisect mask buffer as dispM scratch
                dispM = work.tile([128, 1024], F32, tag=f"msk{b}",
                                  name=f"dm{b}", bufs=1)
                nc.vector.scalar_tensor_tensor(
                    out=dispM[:], in0=logits_A[:, sl], scalar=theta[b][:],
                    in1=sig_A[:, sl], op0=AO.is_gt, op1=AO.mult)
                dispO = work.tile([128, 1024], F32R, tag=f"do{b}",
                                  name=f"do{b}", bufs=1)
                nc.vector.tensor_scalar(out=dispO[:], in0=dispM[:],
                                        scalar1=DSCALE, scalar2=DFLOOR,
                                        op0=AO.mult, op1=AO.add)
                nc.sync.dma_start(out=disp_d[b], in_=dispO[:].bitcast(F32))
                # combine = disp / rowsum_e(disp)
                comb = work.tile([128, 1024], F32, tag=f"cb{b}", name=f"cb{b}",
                                 bufs=1)
                den = work.tile([128, 1024], F32, tag=f"dn{b}", name=f"dn{b}",
                                bufs=1)
                for hhalf in range(2):
                    hsl = slice(hhalf * 512, (hhalf + 1) * 512)
                    p_rep = ps.tile([128, 512], F32, tag="ph", name="p_rep",
                                    bufs=3)
                    nc.tensor.matmul(p_rep[:], mrep_sb[:], dispO[:, hsl],
                                     start=True, stop=True)
                    nc.vector.reciprocal(den[:, hsl], p_rep[:])
                    nc.vector.tensor_tensor(out=comb[:, hsl],
                                            in0=dispO[:, hsl].bitcast(F32),
                                            in1=den[:, hsl], op=AO.mult)
                nc.sync.dma_start(out=comb_d[b], in_=comb[:])

            mlp_batch(0)
            mlp_batch(1)
            finalize(0)
            bisect(0)
            emit(0)
            finalize(1)
            bisect(1)
            emit(1)
            if debug:
                nc.sync.dma_start(out=dbg_dists_d[:], in_=dists_A[:])
                nc.sync.dma_start(out=dbg_logits_d[:], in_=logits_A[:])

    nc.finalize()
    return nc


def _get_prog(debug=False):
    key = ("prog", debug)
    if key not in _prog_cache:
        _prog_cache[key] = _build(debug)
    return _prog_cache[key]


def make_in_maps(inputs):
    tokens = np.asarray(inputs["tokens"], dtype=np.float32)
    xyz = np.asarray(inputs["spatial_xyz"], dtype=np.float32)
    W1 = np.ascontiguousarray(np.asarray(inputs["W1"], dtype=np.float32))
    b1 = np.asarray(inputs["b1"], dtype=np.float32)
    W2 = np.asarray(inputs["W2"], dtype=np.float32)
    b2 = np.asarray(inputs["b2"], dtype=np.float32)
    centers = np.asarray(inputs["centers"], dtype=np.float32)

    # shared constants
    W2p = np.zeros((128, 32), np.float32)
    for mc in range(2):
        W2p[:, mc * 16:(mc + 1) * 16] = W2[mc * 128:(mc + 1) * 128, :]
    b1p = np.ascontiguousarray(b1.reshape(2, 128).T)
    b2bc = np.ascontiguousarray(np.tile(b2, 8)[:, None].astype(np.float32))
    m2 = np.ascontiguousarray(
        (np.arange(128)[:, None] % 16 == np.arange(128)[None, :] % 16)
        .astype(np.float32))
    mrep = np.ascontiguousarray(
        (np.arange(128)[:, None] // 16 == np.arange(128)[None, :] // 16)
        .astype(np.float32))
    # C5 [40, 128]: col 16g+e, rows 5g+c = [-2cx, -2cy, -2cz, 1, |c|^2]
    C5 = np.zeros((40, 128), np.float32)
    csq = (centers.astype(np.float64) ** 2).sum(-1).astype(np.float32)
    for g in range(8):
        for e in range(E):
            p = 16 * g + e
            C5[5 * g + 0, p] = -2.0 * centers[e, 0]
            C5[5 * g + 1, p] = -2.0 * centers[e, 1]
            C5[5 * g + 2, p] = -2.0 * centers[e, 2]
            C5[5 * g + 3, p] = 1.0
            C5[5 * g + 4, p] = csq[e]

    in_maps = []
    for c in range(N_CORES):
        tokT = np.empty((BPC, 515, N), np.float32)
        xp = np.empty((40, 2048), np.float32)
        for b in range(BPC):
            gb = BPC * c + b
            tokT[b, :512] = tokens[gb].T
            tokT[b, 512:515] = xyz[gb].T
            x2 = (xyz[gb].astype(np.float64) ** 2).sum(-1).astype(np.float32)
            for blk in range(2):
                i = 2 * b + blk
                # rows 5g+c over tokens (g, t) of this blk
                xs = xyz[gb][(blk * 8) * 512:(blk * 8 + 8) * 512]  # (8*512, 3)
                xs = xs.reshape(8, 512, 3)
                x2s = x2[(blk * 8) * 512:(blk * 8 + 8) * 512].reshape(8, 512)
                blkm = np.empty((8, 5, 512), np.float32)
                blkm[:, 0:3] = xs.transpose(0, 2, 1)
                blkm[:, 3] = x2s
                blkm[:, 4] = 1.0
                xp[:, i * 512:(i + 1) * 512] = blkm.reshape(40, 512)
        in_maps.append({
            "tokT": np.ascontiguousarray(tokT),
            "xp": np.ascontiguousarray(xp),
            "C5": C5, "W1p": W1, "W2p": W2p, "b1p": b1p, "b2bc": b2bc,
            "m2": m2, "mrep": mrep,
        })
    return in_maps


def _unlayout(arr):
    """[128, 1024] (p=16g+e, f=blk*512+t) -> (8192, 16)"""
    return (arr.reshape(8, 16, 2, 512).transpose(2, 0, 3, 1)
            .reshape(N, E))


def kernel(**inputs):
    from concourse.bass_utils import run_bass_kernel_spmd

    nc = _get_prog(_DEBUG)
    in_maps = make_in_maps(inputs)
    res = run_bass_kernel_spmd(nc, in_maps, list(range(N_CORES)))
    dispatch = np.empty((B, N, E), np.float32)
    combine = np.empty((B, N, E), np.float32)
    for c in range(N_CORES):
        for b in range(BPC):
            dispatch[BPC * c + b] = _unlayout(res.results[c]["disp"][b])
            combine[BPC * c + b] = _unlayout(res.results[c]["comb"][b])
    if _DEBUG:
        kernel._dbg = [(res.results[c]["dbg_logits"], res.results[c]["dbg_dists"])
                       for c in range(N_CORES)]
    return dispatch, combine
